# revision 7
# baseline (speedup 1.0000x reference)
"""Trainium2 Bass kernel for AdaptiveAdjacencyMatrix.

Math: reference computes S = renorm(mask * softmax_j(proj_i + proj_j + b))
with proj = h @ w.  Inside a row softmax the proj_i and b terms cancel, so
every valid row i < size_b of S[b] is the SAME vector
    v_b[j] = exp(proj_j) * mask_j / sum_j'(exp(proj_j') * mask_j')
and rows i >= size_b are zero.  The kernel therefore computes, per batch:
a matvec (PE), one exp with accumulated Z (ACT), a broadcast outer-product
(PE), 1/Z-scaled PSUM->SBUF casts (DVE+ACT), and row-masked scaled copies
(DVE) into the (1024, 1024) output block.

The column mask is folded into h on the host: invalid columns j >= size_b
are replaced by (-30/||w||^2)*w so their projection is -30 and exp ~ 0.
Normalization by 1/Z is folded into the PSUM->SBUF casts (1/Z broadcast to
all partitions via a tiny fp32 matmul + reciprocal), so eb holds the final
row vector v and stores never wait on a separate normalize pass.  Since
sizes >= M/2 by construction, row tiles 0-3 are always fully valid and are
stored straight from eb with a stride-0 broadcast-AP DMA (1MB per batch
with no materialization); tiles 4-7 get per-partition 0/1 mask scalars.

Sharding: pure data-parallel over batch B=32 across 8 cores (4 per core).
No collectives.
"""

import numpy as np

_CORES = 8
_B, _M, _H = 32, 1024, 512
_BLOC = _B // _CORES  # 4 batches per core
_NCHUNK = _H // 128  # 4 contraction chunks
_NROWT = _M // 128  # 8 row tiles per batch
_NHALF = 2  # two 512-col halves per row

_cache = {}


def _get_nc():
    if "nc" in _cache:
        return _cache["nc"]

    import concourse.bacc as bacc
    import concourse.mybir as mybir
    import concourse.tile as tile

    f32 = mybir.dt.float32
    DT = mybir.dt.bfloat16
    Exp = mybir.ActivationFunctionType.Exp

    nc = bacc.Bacc(
        "TRN2",
        target_bir_lowering=False,
        debug=False,
        enable_partition_id=False,
    )

    # hT pre-swizzled on host to [128, chunk, 1024] per batch: partition p =
    # contraction sub-index, so batch 0 loads as 4 per-chunk DMAs and each
    # chunk's matmuls start as soon as that chunk lands (load/matvec pipeline)
    hT_ext = nc.declare_dram_parameter(
        "hT", [_BLOC, 128, _NCHUNK, _M], DT, isOutput=False
    )
    # packed consts: [w4 bf16 (4) | maskpt f32 as bf16 pairs (64)] per
    # partition -> ONE 136B-per-partition load instead of ~1150 tiny
    # (8B/128B) descriptors competing with the h-load ramp
    cext = nc.declare_dram_parameter(
        "cpk", [128, _NCHUNK + 2 * _BLOC * _NROWT], DT, isOutput=False
    )
    # out stored as [bi, p, t, j] (p=partition, t=row tile): fully
    # contiguous per-partition DMA writes; host transposes back
    out_ext = nc.declare_dram_parameter(
        "out", [_BLOC, 128, _NROWT, _M], DT, isOutput=True
    )

    with tile.TileContext(nc) as tc:
        with (
            tc.tile_pool(name="const", bufs=1) as const_pool,
            tc.tile_pool(name="hbuf", bufs=4) as h_pool,
            tc.tile_pool(name="obuf", bufs=4) as out_pool,
            tc.tile_pool(name="vbuf", bufs=2) as v_pool,
            tc.tile_pool(name="small", bufs=4) as small_pool,
            tc.tile_pool(name="psp", bufs=2, space="PSUM") as psum_proj,
            tc.tile_pool(name="pso", bufs=2, space="PSUM") as psum_out,
            tc.tile_pool(name="psz", bufs=2, space="PSUM") as psum_z,
        ):
            cpk_sb = const_pool.tile([128, _NCHUNK + 2 * _BLOC * _NROWT], DT)
            nc.scalar.dma_start(cpk_sb[:], cext[:])
            w_sb = cpk_sb[:, 0 : _NCHUNK]
            maskpt_sb = cpk_sb[
                :, _NCHUNK : _NCHUNK + 2 * _BLOC * _NROWT
            ].bitcast(f32)
            ones_sb = const_pool.tile([1, 128], DT)
            nc.vector.memset(ones_sb[:], 1.0)

            # ---- issue all input loads up front on the sync HWDGE ring.
            # Batch 0 is loaded as four 256KB chunk DMAs (at forced-first
            # priority, so the scheduler cannot reorder full-batch loads
            # ahead of them) so chunk-c matmuls overlap the chunk-c+1 load.
            # Keeping every DMA on the single sync ring measured fastest:
            # SWDGE (gpsimd) stores and scalar-ring load splitting both
            # regressed. ----
            h_tiles = []
            for bi in range(_BLOC):
                hT_t = h_pool.tile([128, _NCHUNK, _M], DT)
                h_tiles.append(hT_t)
            with tc.high_priority():
                for c in range(_NCHUNK):
                    nc.sync.dma_start(h_tiles[0][:, c], hT_ext[0, :, c])
            for bi in range(1, _BLOC):
                nc.sync.dma_start(h_tiles[bi][:], hT_ext[bi])

            # ---- per batch: full chain, batches pipeline via pools ----
            Copy = mybir.ActivationFunctionType.Copy
            htiles = _NROWT // 2
            for bi in range(_BLOC):
                hT_t = h_tiles[bi]
                e_t = small_pool.tile([1, _M], DT, tag=f"e{bi}")
                zs2 = small_pool.tile([1, 2], f32, tag=f"zs2_{bi}")
                eb_sb = v_pool.tile([128, _M], DT)
                # proj for both halves into one 2-bank PSUM tile; chunk-major
                # matmul order so batch 0 pipelines with its chunk loads and
                # consecutive half-matmuls share one LDWEIGHTS.
                pp = psum_proj.tile([1, _M], f32, tag="proj")
                for c in range(_NCHUNK):
                    for n in range(_NHALF):
                        nc.tensor.matmul(
                            pp[0:1, n * 512 : (n + 1) * 512],
                            w_sb[:, c : c + 1],
                            hT_t[:, c, n * 512 : (n + 1) * 512],
                            start=(c == 0),
                            stop=(c == _NCHUNK - 1),
                        )
                # e = exp(proj) per half (half-0 exp overlaps the half-1
                # stop-matmul); no max-shift needed (|proj| < ~6).
                for n in range(_NHALF):
                    nc.scalar.activation(
                        e_t[0:1, n * 512 : (n + 1) * 512],
                        pp[0:1, n * 512 : (n + 1) * 512],
                        Exp,
                        accum_out=zs2[0:1, n : n + 1],
                    )
                # 1/Z broadcast to all 128 partitions: tiny bf16 matmul
                # (an fp32 matmul costs ~1us of PE as a 2-pass LOW/HIGH
                # pair; bf16 Z costs 0.0011 extra rel err, fine vs 2e-2)
                # then reciprocal (runs concurrently with the e broadcast).
                # The half-Z add fuses with the bf16 cast in one DVE op.
                zsum_bf = small_pool.tile([1, 1], DT, tag="zsb")
                nc.vector.tensor_scalar_add(
                    zsum_bf[:], zs2[0:1, 0:1], zs2[0:1, 1:2]
                )
                zb = psum_z.tile([128, 1], f32, tag="zb")
                nc.tensor.matmul(
                    zb[:], ones_sb[:], zsum_bf[:], start=True, stop=True
                )
                rzb = small_pool.tile([128, 1], f32, tag=f"rz{bi}")
                nc.vector.reciprocal(rzb[:], zb[:])

                # broadcast e to 128 partitions (ones^T @ e), then fold the
                # 1/Z normalization into the PSUM->SBUF casts so eb holds
                # the final row vector v in bf16.
                for n in range(_NHALF):
                    ps = psum_out.tile([128, 512], f32, tag="vb")
                    nc.tensor.matmul(
                        ps[:],
                        ones_sb[:],
                        e_t[0:1, n * 512 : (n + 1) * 512],
                        start=True,
                        stop=True,
                    )
                    dst = eb_sb[:, n * 512 : (n + 1) * 512]
                    if n == 0:
                        nc.vector.tensor_scalar_mul(dst, ps[:], rzb[:])
                    else:
                        nc.scalar.activation(dst, ps[:], Copy, scale=rzb[:])

                # sizes >= M/2 always, so row tiles 0-3 are fully valid in
                # every batch: store them straight from eb via a stride-0
                # broadcast AP (no materialization).  Tiles 4-7 need the 0/1
                # row mask: four DVE scaled copies into an SBUF tile.
                nc.sync.dma_start(
                    out_ext[bi, :, 0:htiles, :],
                    eb_sb[:].unsqueeze(1).to_broadcast((128, htiles, _M)),
                )
                out_b = out_pool.tile([128, htiles, _M], DT, tag="out_b")
                for t in range(htiles, _NROWT):
                    sc = maskpt_sb[:, bi * _NROWT + t : bi * _NROWT + t + 1]
                    nc.vector.tensor_scalar_mul(
                        out_b[:, t - htiles, :], eb_sb[:], sc
                    )
                nc.sync.dma_start(
                    out_ext[bi, :, htiles:_NROWT, :], out_b[:]
                )

    nc.compile()
    _cache["nc"] = nc
    return nc


def _np_dt():
    import ml_dtypes

    return np.dtype(ml_dtypes.bfloat16)


def _ensure_ntff_hook():
    """Install the axon NTFF profiling hook if the image's antenv lacks it.

    Mirrors trn_boot._ntff_profile_via_ctypes: drives NRT profiling via the
    libaxon_pjrt.so C ABI so run_bass_kernel_spmd(trace=True) can report
    exec_time_ns.  No-op if anything is missing.
    """
    import contextlib
    import ctypes
    import os
    import sys
    import types

    try:
        from antenv.axon_hooks import get_axon_ntff_profile_hook

        if get_axon_ntff_profile_hook() is not None:
            return
        have_mod = True
    except ImportError:
        have_mod = False

    so_path = "/opt/axon/libaxon_pjrt.so"
    if not os.path.exists(so_path):
        return
    lib = ctypes.CDLL(so_path)
    if not hasattr(lib, "axon_start_nrt_profile"):
        return
    lib.axon_start_nrt_profile.argtypes = [
        ctypes.POINTER(ctypes.c_int64),
        ctypes.c_size_t,
    ]
    lib.axon_start_nrt_profile.restype = ctypes.c_int64
    lib.axon_stop_nrt_profile.argtypes = [ctypes.c_char_p]
    lib.axon_stop_nrt_profile.restype = ctypes.c_int64

    @contextlib.contextmanager
    def _hook(output_dir, device_ids):
        import jax

        jax.devices()
        if device_ids:
            ids = (ctypes.c_int64 * len(device_ids))(*device_ids)
            rc = lib.axon_start_nrt_profile(ids, len(device_ids))
        else:
            rc = lib.axon_start_nrt_profile(None, 0)
        if rc != 0:
            raise RuntimeError(f"axon_start_nrt_profile rc={rc}")
        try:
            yield
        finally:
            n = lib.axon_stop_nrt_profile(str(output_dir).encode())
            print(f"ntff profile: {n} file(s) written to {output_dir}")

    if have_mod:
        from antenv import axon_hooks

        axon_hooks.set_axon_ntff_profile_hook(_hook)
    else:
        mod = types.ModuleType("antenv.axon_hooks")
        state = {"hook": _hook}
        mod.get_axon_ntff_profile_hook = lambda: state["hook"]
        mod.set_axon_ntff_profile_hook = lambda h: state.__setitem__("hook", h)
        sys.modules["antenv.axon_hooks"] = mod


def _run_with_retry(nc, in_maps, trace, attempts=3):
    """Retry transient device errors (NRT_EXEC_UNIT_UNRECOVERABLE has been
    observed to clear on re-execution)."""
    import time

    from concourse.bass_utils import run_bass_kernel_spmd

    for a in range(attempts):
        try:
            return run_bass_kernel_spmd(
                nc, in_maps, core_ids=list(range(_CORES)), trace=trace
            )
        except Exception:
            if a == attempts - 1:
                raise
            time.sleep(8)


def kernel(h, w, b, original_sizes, _trace=False):
    if _trace:
        _ensure_ntff_hook()
    nc = _get_nc()
    dt = _np_dt()

    h = np.asarray(h, dtype=np.float32)
    w = np.asarray(w, dtype=np.float32)
    sizes = np.asarray(original_sizes).astype(np.int64)

    # Fold the column mask into h: invalid columns j >= size_b project to
    # -30 (so exp ~ 0) by replacing h[b, j, :] with (-30/||w||^2) * w.
    hm = h.copy()
    alpha_w = (-30.0 / float(np.dot(w, w))) * w
    for bb in range(_B):
        hm[bb, int(sizes[bb]) :, :] = alpha_w

    # hT[b, p, c, j] = h[b, j, c*128+p]
    hT = np.ascontiguousarray(
        hm.transpose(0, 2, 1)  # (B, H, M)
        .reshape(_B, _NCHUNK, 128, _M)
        .transpose(0, 2, 1, 3)
    ).astype(dt)
    w4 = np.ascontiguousarray(w.reshape(_NCHUNK, 128).T).astype(dt)  # (128, 4)
    mask = (np.arange(_M)[None, :] < sizes[:, None]).astype(np.float32)  # (B, M)
    # maskpt[p, b*NROWT + t] = mask[b, t*128 + p]
    mask_pt = np.ascontiguousarray(
        mask.reshape(_B, _NROWT, 128).transpose(2, 0, 1).reshape(128, _B * _NROWT)
    )

    def _pack_consts(i):
        mp = np.ascontiguousarray(
            mask_pt[:, i * _BLOC * _NROWT : (i + 1) * _BLOC * _NROWT]
        ).astype(np.float32)
        return np.ascontiguousarray(
            np.concatenate([w4.view(np.uint16), mp.view(np.uint16)], axis=1)
        ).view(dt)

    in_maps = [
        {
            "hT": np.ascontiguousarray(hT[i * _BLOC : (i + 1) * _BLOC]),
            "cpk": _pack_consts(i),
        }
        for i in range(_CORES)
    ]

    res = _run_with_retry(nc, in_maps, trace=_trace)
    _cache["last_result"] = res

    out = np.concatenate(
        [np.asarray(res.results[i]["out"]) for i in range(_CORES)], axis=0
    )  # (B, 128, NROWT, M) with row i = t*128 + p
    out = out.astype(np.float32).transpose(0, 2, 1, 3).reshape(_B, _M, _M)
    return np.ascontiguousarray(out)


def last_exec_time_ns():
    res = _cache.get("last_result")
    return None if res is None else res.exec_time_ns

